# revision 23
# baseline (speedup 1.0000x reference)
"""CTC loss (Keras ctc_batch_cost semantics) on 8 Trainium2 NeuronCores.

Strategy
--------
Data-parallel over batch: core q handles examples [32q, 32q+32).

The CTC alpha recursion runs in *linear probability space*, made feasible
for bf16/fp32 exponent range by three transforms:

1. Static exponential position equilibration: state stored as
   G[j] = g[j]*e^{-c*j}, D[i] = d[i]*e^{-c*(i-1)} with c = 3.5 (the
   measured lattice tilt in nats/position).  Folds entirely into the ksm
   mask, one scalar constant in the step, the D init value, and two
   scalar epilogue constants.
2. Blank-prob folding: the host pre-divides all probs by the per-frame
   blank prob, which turns the blank-chain update into a PURE ADD
   (D1 = D0 + shift(h)); sum(ln p_blank) is restored via one
   Ln-with-accumulate instruction over a [64, 512] input.
3. A single per-row floating rescale (by the D row max) every 64 steps.

Time is split forward/backward (meet in the middle): rows 0-31 run alpha
forward over t = 0..511, rows 32-63 run the suffix recursion backward
over t = 1023..512 in reversed label coordinates.

Per time step (bf16 state, 64 rows on partitions, ~129 free), emitted
in the order h, kh, t1, g1 so kh (a 4x op) runs before GpSimd's D1
saturates the SBUF ports DVE shares with it, and the D0 semaphore wait
hides under kh:
    h  = g0 * pl                   TT   DVE (4x mode)
    kh = h * ksm~                  TT   DVE (4x)
    t1 = (D0 * e^-c) + h           STT  DVE (const scalar, 1x)
    g1 = t1 + shift1(kh)           TT   DVE (4x)
    D1 = D0 + shift1(h)            TT   GpSimd (runs in parallel)
Shifts are contiguous APs via zero guard columns; guard offsets are
chosen so every DVE *read* starts at an even bf16 element (alignment
keeps the DVE fast mode).  Step period ~1.05us traced; the whole
recursion is the critical path, everything else overlaps it.

Per-step symbol probabilities are gathered on TensorE as one-hot matmuls
in bf16 (Sel matrices prebuilt on host, 128 label columns only), bounced
through DRAM to transpose [time-major] -> [row-major], all in bf16.
ScalarE does the PSUM->SBUF copies.  First chunks are small (32/96) to
cut pipeline-fill latency before the serial chain can start.
"""
import sys
from contextlib import ExitStack

import numpy as np
import ml_dtypes

# ---------------- problem constants (hardcoded per the task spec) -------------
B, T, C, L = 256, 1024, 256, 128
NCORES = 8
NE = B // NCORES          # examples per core (32)
VR = 2 * NE               # virtual rows: fwd + bwd (64)
TH = T // 2               # sequential steps per half (512)
NT = 128                  # time-block (chunk) size
NCHUNK = TH // NT         # 4
SW = 128                  # gathered columns: labels only (probs pre-divided by blank)
RB = 64                   # rescale interval
NR = TH // RB             # number of rescales (16)
EPS = 1e-7
CEQ = 3.5                 # equilibration nats/position
TINY = 1e-37
NEG = -1e18

_TRN_REPO = "/opt/trn_rl_repo"
BF16NP = ml_dtypes.bfloat16


def _ensure_path():
    if _TRN_REPO not in sys.path:
        sys.path.insert(0, _TRN_REPO)


# ----------------------------- device kernel ---------------------------------

def build_nc():
    """Build and compile the Bass module (same NEFF for all 8 cores)."""
    _ensure_path()
    import concourse.bass as bass
    import concourse.mybir as mybir
    import concourse.tile as tile
    from concourse import bacc

    f32 = mybir.dt.float32
    bf16 = mybir.dt.bfloat16
    i32 = mybir.dt.int32
    Alu = mybir.AluOpType
    Act = mybir.ActivationFunctionType

    nc = bacc.Bacc(
        "TRN2", target_bir_lowering=False, debug=False, num_devices=NCORES
    )

    xpf_d = nc.dram_tensor("xpf", [2, 128, TH, NE], bf16, kind="ExternalInput").ap()
    xpb_d = nc.dram_tensor("xpb", [2, 128, TH, NE], bf16, kind="ExternalInput").ap()
    sel_d = nc.dram_tensor("sel", [128, 2 * VR * SW], bf16, kind="ExternalInput").ap()
    ksm_d = nc.dram_tensor("ksm", [VR, L], bf16, kind="ExternalInput").ap()
    pbln_d = nc.dram_tensor("pbln", [VR, TH], bf16, kind="ExternalInput").ap()
    scr_d = nc.dram_tensor("scr", [2, VR, NT * SW], bf16).ap()
    loss_d = nc.dram_tensor("loss", [NE, 1], f32, kind="ExternalOutput").ap()

    with tile.TileContext(nc) as tc, ExitStack() as ctx:
        const_p = ctx.enter_context(tc.tile_pool(name="const", bufs=1))
        state_p = ctx.enter_context(tc.tile_pool(name="state", bufs=1))
        chunk_p = ctx.enter_context(tc.tile_pool(name="chunk", bufs=2))
        xt_p = ctx.enter_context(tc.tile_pool(name="xt", bufs=2))
        evb_p = ctx.enter_context(tc.tile_pool(name="evb", bufs=2))
        ps_p = ctx.enter_context(
            tc.tile_pool(name="ps", bufs=8, space=bass.MemorySpace.PSUM)
        )
        ep_p = ctx.enter_context(tc.tile_pool(name="ep", bufs=1))

        V, S, G = nc.vector, nc.scalar, nc.gpsimd

        # ---- constants (on the GpSimd-dispatched DMA queue so the sync
        # queue starts chunk-0's x loads immediately — one queue serializes)
        sel_t = const_p.tile([128, 2, VR, SW], bf16, tag="sel")
        sel_src = sel_d.rearrange("c (x v s) -> c x v s", x=2, s=SW)
        # sel first on this queue — it gates the first matmuls; split the
        # load so they start after ~1/8 of it
        for v0 in range(0, VR, 8):
            G.dma_start(
                sel_t[:, :, v0:v0 + 8, :], sel_src[:, :, v0:v0 + 8, :]
            )
        ksm_t = const_p.tile([VR, L], bf16, tag="ksm")
        G.dma_start(ksm_t[:, :], ksm_d[:, :])
        pbln_t = const_p.tile([VR, TH], bf16, tag="pbln")
        G.dma_start(pbln_t[:, :], pbln_d[:, :])
        lnpb = const_p.tile([VR, TH], f32, tag="lnpb")
        lnpb_sum = const_p.tile([VR, 1], f32, tag="lnpb_sum")
        S.activation(lnpb[:, :], pbln_t[:, :], Act.Ln, accum_out=lnpb_sum[:, :])

        # ---- state tiles (bf16), ping-pong pairs
        def zt(tag, w):
            t = state_p.tile([VR, w], bf16, tag=tag)
            V.memset(t[:, :], 0.0)
            return t

        gA, gB = zt("gA", L), zt("gB", L)
        DA, DB = zt("DA", 129), zt("DB", 129)
        # h payload lives at cols 2-129 (even start), cols 0-1 = 0 guard;
        # kh payload at cols 3-130 (written shifted), cols 0-2 = 0 guard —
        # all V-engine READS start at even bf16 offsets (DVE fast mode).
        hA, hB = zt("hA", 130), zt("hB", 130)
        kA, kB = zt("kA", 131), zt("kB", 131)
        tA, tB = zt("tA", L), zt("tB", L)
        utT = state_p.tile([VR, 129], bf16, tag="utT")
        V.memset(gA[:, 0:1], 1.0)
        V.memset(DA[:, 0:1], float(np.exp(CEQ)))

        rmax = state_p.tile([VR, NR], f32, tag="rmax")
        rinv = state_p.tile([VR, 1], f32, tag="rinv")

        cur_g, new_g = gA, gB
        cur_D, new_D = DA, DB
        cur_h, new_h = hA, hB
        cur_k, new_k = kA, kB
        cur_t, new_t = tA, tB

        # ---- main loop (first chunks small to cut pipeline-fill latency)
        CHUNKS = [32, 96, 128, 128, 128]
        assert sum(CHUNKS) == TH
        m0 = 0
        for blk, CNT in enumerate(CHUNKS):
            xts = []
            for d, srcd in enumerate((xpf_d, xpb_d)):
                xb = xt_p.tile([128, 2, NT, NE], bf16, tag=f"xb{d}")
                for cc in range(2):
                    nc.sync.dma_start(
                        xb[:, cc, 0:CNT, :], srcd[cc][:, m0:m0 + CNT, :]
                    )
                xts.append(xb)
            # scr is [v, m, s]-ordered: writes carry the transpose cost
            # (overlapped with the matmuls, in row-groups of 8), the
            # inner-loop-critical plc read is 64 contiguous runs.
            scrb = scr_d[blk % 2].rearrange("v (m s) -> m v s", s=SW)
            for half in range(2):
                evb = evb_p.tile([NT, NE * SW], bf16, tag="evb")
                for vq in range(NE):
                    v = half * NE + vq
                    xb = xts[half]
                    ps = ps_p.tile([NT, SW], f32, tag="gps")
                    nc.tensor.matmul(
                        ps[0:CNT, :], xb[:, 0, 0:CNT, vq], sel_t[:, 0, v, :],
                        start=True, stop=False,
                    )
                    nc.tensor.matmul(
                        ps[0:CNT, :], xb[:, 1, 0:CNT, vq], sel_t[:, 1, v, :],
                        start=False, stop=True,
                    )
                    dst = evb[0:CNT, vq * SW:(vq + 1) * SW]
                    if blk == 0 and vq % 2 == 1:
                        # pre-loop: V is idle — split the PSUM copies so
                        # ScalarE isn't the chunk-0 serializer (GpSimd
                        # cannot access PSUM)
                        V.tensor_copy(dst, ps[0:CNT, :])
                    else:
                        S.copy(dst, ps[0:CNT, :])
                    if vq % 8 == 7:
                        v0 = half * NE + vq - 7
                        nc.sync.dma_start(
                            scrb[0:CNT, v0:v0 + 8, :],
                            evb[0:CNT, (vq - 7) * SW:(vq + 1) * SW],
                        )
            plc = chunk_p.tile([VR, NT * SW], bf16, tag="plc")
            nc.sync.dma_start(
                plc[:, 0:CNT * SW], scr_d[blk % 2][:, 0:CNT * SW]
            )
            for ml in range(CNT):
                m = m0 + ml
                pl = plc[:, ml * SW: ml * SW + 128]
                g0, D0 = cur_g, cur_D
                hN, kN, tN = new_h, new_k, new_t
                g1, D1 = new_g, new_D

                # V: 3 bf16 TT (4x) + 1 const-scalar STT; G: the pure D-add.
                # kh directly after h: runs before GpSimd's D1 saturates the
                # shared SBUF ports, and the D0-sem wait hides under it.
                if m % RB == 0 and m > 0:
                    # g-rescale folded in: h = (g0*rinv)*pl
                    V.scalar_tensor_tensor(
                        hN[:, 2:130], g0[:, :], rinv[:, 0:1], pl,
                        Alu.mult, Alu.mult,
                    )
                else:
                    V.tensor_tensor(hN[:, 2:130], g0[:, :], pl, Alu.mult)
                V.tensor_tensor(kN[:, 3:131], hN[:, 2:130], ksm_t[:, :], Alu.mult)
                V.scalar_tensor_tensor(
                    tN[:, :], D0[:, 0:128], float(np.exp(-CEQ)), hN[:, 2:130],
                    Alu.mult, Alu.add,
                )
                V.tensor_tensor(g1[:, :], tN[:, :], kN[:, 2:130], Alu.add)

                if m == TH - 1:
                    S.copy(utT[:, :], D0[:, :])   # u_T(bar) = D_(T-1)(bar)

                G.tensor_tensor(D1[:, :], D0[:, :], hN[:, 1:130], Alu.add)

                if (m + 1) % RB == 0:
                    r = (m + 1) // RB - 1
                    V.tensor_reduce(
                        rmax[:, r:r + 1], D1[:, :],
                        axis=mybir.AxisListType.X, op=Alu.max,
                    )
                    V.tensor_single_scalar(
                        rmax[:, r:r + 1], rmax[:, r:r + 1], 1e-30, Alu.max
                    )
                    V.reciprocal(rinv[:, :], rmax[:, r:r + 1])
                    if m < TH - 1:
                        # g1 is rescaled lazily inside the next step's h op
                        pass
                    else:
                        V.tensor_single_scalar(
                            g1[:, :], g1[:, :], rinv[:, 0:1], Alu.mult
                        )
                    V.tensor_single_scalar(D1[:, :], D1[:, :], rinv[:, 0:1], Alu.mult)
                    if m == TH - 1:
                        V.tensor_single_scalar(
                            hN[:, 2:130], hN[:, 2:130], rinv[:, 0:1], Alu.mult
                        )
                        V.tensor_single_scalar(
                            utT[:, :], utT[:, :], rinv[:, 0:1], Alu.mult
                        )

                cur_g, new_g = new_g, cur_g
                cur_D, new_D = new_D, cur_D
                cur_h, new_h = new_h, cur_h
                cur_k, new_k = new_k, cur_k
                cur_t, new_t = new_t, cur_t
            m0 += CNT

        # ---- epilogue: log-space combine
        # fwd rows (0:NE): aL = h_final, aB = ut_final
        # bwd rows (NE:VR): bL = g_final, bB = D_final
        hF = cur_h    # last written h
        gF, DF = cur_g, cur_D

        hf32 = ep_p.tile([VR, L], f32, tag="hf32")
        uf32 = ep_p.tile([VR, 129], f32, tag="uf32")
        gf32 = ep_p.tile([VR, L], f32, tag="gf32")
        Df32 = ep_p.tile([VR, 129], f32, tag="Df32")
        S.copy(hf32[:, :], hF[:, 2:130])
        S.copy(uf32[:, :], utT[:, :])
        S.copy(gf32[:, :], gF[:, :])
        S.copy(Df32[:, :], DF[:, :])

        _sln_n = [0]

        def safe_ln(dst_ap, src_ap, n, rows=slice(0, VR)):
            """dst = ln(src) via exponent extraction; HW Ln table is only
            accurate on ~[1e-10, 2^64]; mantissa lives in [1, 2).
            Fresh scratch tiles per call so independent calls overlap."""
            _sln_n[0] += 1
            u = _sln_n[0]
            ii_t = ep_p.tile([VR, n], i32, tag=f"sln_i{u}")
            mm_t = ep_p.tile([VR, n], i32, tag=f"sln_m{u}")
            ee_t = ep_p.tile([VR, n], f32, tag=f"sln_e{u}")
            ll_t = ep_p.tile([VR, n], f32, tag=f"sln_l{u}")
            ii, mm, ee, ll = ii_t[rows, :], mm_t[rows, :], ee_t[rows, :], ll_t[rows, :]
            V.tensor_single_scalar(ii, src_ap.bitcast(i32), 23, Alu.arith_shift_right)
            V.tensor_single_scalar(ii, ii, 127, Alu.subtract)
            V.tensor_single_scalar(mm, src_ap.bitcast(i32), 0x007FFFFF, Alu.bitwise_and)
            V.tensor_single_scalar(mm, mm, 0x3F800000, Alu.bitwise_or)
            S.activation(ll, mm.bitcast(f32), Act.Ln)
            V.tensor_copy(ee, ii)
            V.scalar_tensor_tensor(
                dst_ap, ee, 0.6931471805599453, ll, Alu.mult, Alu.add
            )

        # lZ = sum of ln(rmax) over the 16 rescales
        lnr = ep_p.tile([VR, NR], f32, tag="lnr")
        lZ = ep_p.tile([VR, 1], f32, tag="lZ")
        safe_ln(lnr[:, :], rmax[:, :], NR)
        V.tensor_reduce(lZ[:, :], lnr[:, :], axis=mybir.AxisListType.X, op=Alu.add)
        V.tensor_tensor(lZ[:, :], lZ[:, :], lnpb_sum[:, :], Alu.add)

        LL = ep_p.tile([VR, L], f32, tag="LL")
        LB = ep_p.tile([VR, 129], f32, tag="LB")
        cl1 = ep_p.tile([VR, L], f32, tag="cl1")
        eq1 = ep_p.tile([VR, L], f32, tag="eq1")
        ln1 = ep_p.tile([VR, L], f32, tag="ln1")
        cl2 = ep_p.tile([VR, 129], f32, tag="cl2")
        eq2 = ep_p.tile([VR, 129], f32, tag="eq2")
        ln2 = ep_p.tile([VR, 129], f32, tag="ln2")

        # bwd rows first so the reversal DMAs start while fwd rows compute
        V.tensor_single_scalar(cl1[NE:VR, :], gf32[NE:VR, :], TINY, Alu.max)
        V.tensor_single_scalar(eq1[NE:VR, :], gf32[NE:VR, :], 0.0, Alu.is_equal)
        V.tensor_single_scalar(cl2[NE:VR, :], Df32[NE:VR, :], TINY, Alu.max)
        V.tensor_single_scalar(eq2[NE:VR, :], Df32[NE:VR, :], 0.0, Alu.is_equal)
        safe_ln(ln1[NE:VR, :], cl1[NE:VR, :], L, rows=slice(NE, VR))
        safe_ln(ln2[NE:VR, :], cl2[NE:VR, :], 129, rows=slice(NE, VR))
        V.scalar_tensor_tensor(
            LL[NE:VR, :], eq1[NE:VR, :], NEG, ln1[NE:VR, :], Alu.mult, Alu.add
        )
        V.scalar_tensor_tensor(
            LB[NE:VR, :], eq2[NE:VR, :], NEG, ln2[NE:VR, :], Alu.mult, Alu.add
        )
        V.tensor_single_scalar(LL[NE:VR, :], LL[NE:VR, :], lZ[NE:VR, 0:1], Alu.add)
        V.tensor_single_scalar(LB[NE:VR, :], LB[NE:VR, :], lZ[NE:VR, 0:1], Alu.add)

        LLb = ep_p.tile([NE, L], f32, tag="LLb")
        LBb = ep_p.tile([NE, 129], f32, tag="LBb")
        nc.sync.dma_start(LLb[:, :], LL[NE:VR, ::-1])
        nc.sync.dma_start(LBb[:, :], LB[NE:VR, ::-1])

        V.tensor_single_scalar(cl1[0:NE, :], hf32[0:NE, :], TINY, Alu.max)
        V.tensor_single_scalar(eq1[0:NE, :], hf32[0:NE, :], 0.0, Alu.is_equal)
        V.tensor_single_scalar(cl2[0:NE, :], uf32[0:NE, :], TINY, Alu.max)
        V.tensor_single_scalar(eq2[0:NE, :], uf32[0:NE, :], 0.0, Alu.is_equal)
        safe_ln(ln1[0:NE, :], cl1[0:NE, :], L, rows=slice(0, NE))
        safe_ln(ln2[0:NE, :], cl2[0:NE, :], 129, rows=slice(0, NE))
        V.scalar_tensor_tensor(
            LL[0:NE, :], eq1[0:NE, :], NEG, ln1[0:NE, :], Alu.mult, Alu.add
        )
        V.scalar_tensor_tensor(
            LB[0:NE, :], eq2[0:NE, :], NEG, ln2[0:NE, :], Alu.mult, Alu.add
        )
        V.tensor_single_scalar(LL[0:NE, :], LL[0:NE, :], lZ[0:NE, 0:1], Alu.add)
        V.tensor_single_scalar(LB[0:NE, :], LB[0:NE, :], lZ[0:NE, 0:1], Alu.add)

        lPL = ep_p.tile([NE, L], f32, tag="lPL")
        lPB = ep_p.tile([NE, 129], f32, tag="lPB")
        V.tensor_tensor(lPL[:, :], LL[0:NE, :], LLb[:, :], Alu.add)
        V.tensor_tensor(lPB[:, :], LB[0:NE, :], LBb[:, :], Alu.add)
        # equilibration pairing constants
        V.tensor_single_scalar(lPL[:, :], lPL[:, :], 127.0 * CEQ, Alu.add)
        V.tensor_single_scalar(lPB[:, :], lPB[:, :], 126.0 * CEQ, Alu.add)

        m1 = ep_p.tile([NE, 1], f32, tag="m1")
        m2 = ep_p.tile([NE, 1], f32, tag="m2")
        V.tensor_reduce(m1[:, :], lPL[:, :], axis=mybir.AxisListType.X, op=Alu.max)
        V.tensor_reduce(m2[:, :], lPB[:, :], axis=mybir.AxisListType.X, op=Alu.max)
        V.tensor_tensor(m1[:, :], m1[:, :], m2[:, :], Alu.max)
        sm1 = ep_p.tile([NE, L], f32, tag="sm1")
        sm2 = ep_p.tile([NE, 129], f32, tag="sm2")
        V.tensor_single_scalar(sm1[:, :], lPL[:, :], m1[:, 0:1], Alu.subtract)
        V.tensor_single_scalar(sm2[:, :], lPB[:, :], m1[:, 0:1], Alu.subtract)
        e1 = ep_p.tile([NE, L], f32, tag="e1")
        e2 = ep_p.tile([NE, 129], f32, tag="e2")
        s1 = ep_p.tile([NE, 1], f32, tag="s1")
        s2 = ep_p.tile([NE, 1], f32, tag="s2")
        S.activation(e1[:, :], sm1[:, :], Act.Exp, accum_out=s1[:, :])
        S.activation(e2[:, :], sm2[:, :], Act.Exp, accum_out=s2[:, :])
        V.tensor_tensor(s1[:, :], s1[:, :], s2[:, :], Alu.add)
        lsum = ep_p.tile([NE, 1], f32, tag="lsum")
        S.activation(lsum[:, :], s1[:, :], Act.Ln)
        lossT = ep_p.tile([NE, 1], f32, tag="lossT")
        V.tensor_tensor(lossT[:, :], m1[:, :], lsum[:, :], Alu.add)
        V.tensor_single_scalar(lossT[:, :], lossT[:, :], -1.0, Alu.mult)
        nc.sync.dma_start(loss_d[:, :], lossT[:, :])

    nc.compile()
    return nc


# ------------------------------- host side ------------------------------------

def make_inputs(y_true, y_pred):
    """Build the 8 per-core input maps (all bf16).

    Probs are pre-divided by the per-frame blank prob so the on-device D
    recursion is a pure add; sum(ln pb) is accounted via the pbln input.
    """
    y_true = np.asarray(y_true)
    y_pred = np.asarray(y_pred, dtype=np.float32)
    p = y_pred + np.float32(EPS)                           # [B, T, C]
    pb = p[:, :, C - 1:C]                                  # [B, T, 1]
    xq = (p / pb).astype(BF16NP)                           # ratio probs
    pbq = np.ascontiguousarray(pb[:, :, 0]).astype(BF16NP) # [B, T]

    lab = y_true.astype(np.int64)                          # [B, L]
    k = np.ones((B, L), np.float32)
    k[:, 1:] = (lab[:, 1:] != lab[:, :-1]).astype(np.float32)
    ks = np.ones((B, L), np.float32)
    ks[:, :-1] = k[:, 1:]
    e_c = np.float32(np.exp(-CEQ))

    cgrid = np.arange(128, dtype=np.int64)

    in_maps = []
    for q in range(NCORES):
        sl = slice(q * NE, (q + 1) * NE)
        pq = xq[sl]                                         # [NE, T, C]
        # layout [cc, c', t, e]: chunk DMAs become contiguous 8KB runs
        xpf = np.ascontiguousarray(
            pq[:, :TH, :].transpose(2, 1, 0).reshape(2, 128, TH, NE)
        )
        xpb = np.ascontiguousarray(
            pq[:, TH:, :][:, ::-1, :].transpose(2, 1, 0).reshape(2, 128, TH, NE)
        )

        pbln = np.empty((VR, TH), np.float32)
        pbln[:NE] = pbq[sl][:, :TH]
        pbln[NE:] = pbq[sl][:, TH:][:, ::-1]
        pbln = pbln.astype(BF16NP)

        labext = np.empty((VR, SW), np.int64)
        labext[:NE, :L] = lab[sl]
        labext[NE:, :L] = lab[sl][:, ::-1]
        # sel[c', cc, v, s] = (labext[v,s] == c' + 128*cc)
        sel = (
            labext[None, None, :, :] == (cgrid[:, None, None, None] + 128 * np.arange(2)[None, :, None, None])
        ).astype(np.float32)
        sel = np.ascontiguousarray(sel.reshape(128, 2 * VR * SW).astype(BF16NP))

        ksm = np.empty((VR, L), np.float32)
        ksm[:NE] = ks[sl]
        ksm[NE:] = k[sl][:, ::-1]
        ksm = (ksm * e_c).astype(BF16NP)

        in_maps.append({
            "xpf": xpf,
            "xpb": xpb,
            "sel": sel,
            "ksm": ksm,
            "pbln": pbln,
        })
    return in_maps


_NC_CACHE = {}


def _get_nc():
    if "nc" not in _NC_CACHE:
        _NC_CACHE["nc"] = build_nc()
    return _NC_CACHE["nc"]


def kernel(y_true, y_pred):
    _ensure_path()
    from concourse.bass_utils import run_bass_kernel_spmd

    nc = _get_nc()
    in_maps = make_inputs(y_true, y_pred)
    res = run_bass_kernel_spmd(nc, in_maps, core_ids=list(range(NCORES)))
    loss = np.concatenate([r["loss"] for r in res.results], axis=0)
    return loss.astype(np.float32)


if __name__ == "__main__":
    nc = build_nc()
    print("built + compiled OK")


# revision 24
# speedup vs baseline: 1.0038x; 1.0038x over previous
"""CTC loss (Keras ctc_batch_cost semantics) on 8 Trainium2 NeuronCores.

Strategy
--------
Data-parallel over batch: core q handles examples [32q, 32q+32).

The CTC alpha recursion runs in *linear probability space*, made feasible
for bf16/fp32 exponent range by three transforms:

1. Static exponential position equilibration: state stored as
   G[j] = g[j]*e^{-c*j}, D[i] = d[i]*e^{-c*(i-1)} with c = 3.5 (the
   measured lattice tilt in nats/position).  Folds entirely into the ksm
   mask, one scalar constant in the step, the D init value, and two
   scalar epilogue constants.
2. Blank-prob folding: the host pre-divides all probs by the per-frame
   blank prob, which turns the blank-chain update into a PURE ADD
   (D1 = D0 + shift(h)); sum(ln p_blank) is restored via one
   Ln-with-accumulate instruction over a [64, 512] input.
3. A single per-row floating rescale (by the D row max) every 64 steps.

Time is split forward/backward (meet in the middle): rows 0-31 run alpha
forward over t = 0..511, rows 32-63 run the suffix recursion backward
over t = 1023..512 in reversed label coordinates.

Per time step (bf16 state, 64 rows on partitions, ~129 free), emitted
in the order h, kh, t1, g1 so kh (a 4x op) runs before GpSimd's D1
saturates the SBUF ports DVE shares with it, and the D0 semaphore wait
hides under kh:
    h  = g0 * pl                   TT   DVE (4x mode)
    kh = h * ksm~                  TT   DVE (4x)
    t1 = (D0 * e^-c) + h           STT  DVE (const scalar, 1x)
    g1 = t1 + shift1(kh)           TT   DVE (4x)
    D1 = D0 + shift1(h)            TT   GpSimd (runs in parallel)
Shifts are contiguous APs via zero guard columns; guard offsets are
chosen so every DVE *read* starts at an even bf16 element (alignment
keeps the DVE fast mode).  Step period ~1.05us traced; the whole
recursion is the critical path, everything else overlaps it.

Per-step symbol probabilities are gathered on TensorE as one-hot matmuls
in bf16 (Sel matrices prebuilt on host, 128 label columns only), bounced
through DRAM to transpose [time-major] -> [row-major], all in bf16.
ScalarE does the PSUM->SBUF copies.  First chunks are small (32/96) to
cut pipeline-fill latency before the serial chain can start.
"""
import sys
from contextlib import ExitStack

import numpy as np
import ml_dtypes

# ---------------- problem constants (hardcoded per the task spec) -------------
B, T, C, L = 256, 1024, 256, 128
NCORES = 8
NE = B // NCORES          # examples per core (32)
VR = 2 * NE               # virtual rows: fwd + bwd (64)
TH = T // 2               # sequential steps per half (512)
NT = 128                  # time-block (chunk) size
NCHUNK = TH // NT         # 4
SW = 128                  # gathered columns: labels only (probs pre-divided by blank)
RB = 64                   # rescale interval
NR = TH // RB             # number of rescales (16)
EPS = 1e-7
CEQ = 3.5                 # equilibration nats/position
TINY = 1e-37
NEG = -1e18

_TRN_REPO = "/opt/trn_rl_repo"
BF16NP = ml_dtypes.bfloat16


def _ensure_path():
    if _TRN_REPO not in sys.path:
        sys.path.insert(0, _TRN_REPO)


# ----------------------------- device kernel ---------------------------------

def build_nc():
    """Build and compile the Bass module (same NEFF for all 8 cores)."""
    _ensure_path()
    import concourse.bass as bass
    import concourse.mybir as mybir
    import concourse.tile as tile
    from concourse import bacc

    f32 = mybir.dt.float32
    bf16 = mybir.dt.bfloat16
    i32 = mybir.dt.int32
    Alu = mybir.AluOpType
    Act = mybir.ActivationFunctionType

    nc = bacc.Bacc(
        "TRN2", target_bir_lowering=False, debug=False, num_devices=NCORES
    )

    xpf_d = nc.dram_tensor("xpf", [2, 128, TH, NE], bf16, kind="ExternalInput").ap()
    xpb_d = nc.dram_tensor("xpb", [2, 128, TH, NE], bf16, kind="ExternalInput").ap()
    sel_d = nc.dram_tensor("sel", [128, 2 * VR * SW], bf16, kind="ExternalInput").ap()
    ksm_d = nc.dram_tensor("ksm", [VR, L], bf16, kind="ExternalInput").ap()
    pbln_d = nc.dram_tensor("pbln", [VR, TH], bf16, kind="ExternalInput").ap()
    scr_d = nc.dram_tensor("scr", [2, VR, NT * SW], bf16).ap()
    loss_d = nc.dram_tensor("loss", [NE, 1], f32, kind="ExternalOutput").ap()

    with tile.TileContext(nc) as tc, ExitStack() as ctx:
        const_p = ctx.enter_context(tc.tile_pool(name="const", bufs=1))
        state_p = ctx.enter_context(tc.tile_pool(name="state", bufs=1))
        chunk_p = ctx.enter_context(tc.tile_pool(name="chunk", bufs=2))
        xt_p = ctx.enter_context(tc.tile_pool(name="xt", bufs=2))
        evb_p = ctx.enter_context(tc.tile_pool(name="evb", bufs=2))
        ps_p = ctx.enter_context(
            tc.tile_pool(name="ps", bufs=8, space=bass.MemorySpace.PSUM)
        )
        ep_p = ctx.enter_context(tc.tile_pool(name="ep", bufs=1))

        V, S, G = nc.vector, nc.scalar, nc.gpsimd

        # ---- constants (on the GpSimd-dispatched DMA queue so the sync
        # queue starts chunk-0's x loads immediately — one queue serializes)
        ksm_t = const_p.tile([VR, L], bf16, tag="ksm")
        G.dma_start(ksm_t[:, :], ksm_d[:, :])
        pbln_t = const_p.tile([VR, TH], bf16, tag="pbln")
        G.dma_start(pbln_t[:, :], pbln_d[:, :])
        lnpb = const_p.tile([VR, TH], f32, tag="lnpb")
        lnpb_sum = const_p.tile([VR, 1], f32, tag="lnpb_sum")
        S.activation(lnpb[:, :], pbln_t[:, :], Act.Ln, accum_out=lnpb_sum[:, :])
        sel_t = const_p.tile([128, 2, VR, SW], bf16, tag="sel")
        sel_src = sel_d.rearrange("c (x v s) -> c x v s", x=2, s=SW)
        # split the load so the first matmuls start after ~1/8 of the DMA
        for v0 in range(0, VR, 8):
            G.dma_start(
                sel_t[:, :, v0:v0 + 8, :], sel_src[:, :, v0:v0 + 8, :]
            )

        # ---- state tiles (bf16), ping-pong pairs
        def zt(tag, w):
            t = state_p.tile([VR, w], bf16, tag=tag)
            V.memset(t[:, :], 0.0)
            return t

        gA, gB = zt("gA", L), zt("gB", L)
        DA, DB = zt("DA", 129), zt("DB", 129)
        # h payload lives at cols 2-129 (even start), cols 0-1 = 0 guard;
        # kh payload at cols 3-130 (written shifted), cols 0-2 = 0 guard —
        # all V-engine READS start at even bf16 offsets (DVE fast mode).
        hA, hB = zt("hA", 130), zt("hB", 130)
        kA, kB = zt("kA", 131), zt("kB", 131)
        tA, tB = zt("tA", L), zt("tB", L)
        utT = state_p.tile([VR, 129], bf16, tag="utT")
        V.memset(gA[:, 0:1], 1.0)
        V.memset(DA[:, 0:1], float(np.exp(CEQ)))

        rmax = state_p.tile([VR, NR], f32, tag="rmax")
        rinv = state_p.tile([VR, 1], f32, tag="rinv")

        cur_g, new_g = gA, gB
        cur_D, new_D = DA, DB
        cur_h, new_h = hA, hB
        cur_k, new_k = kA, kB
        cur_t, new_t = tA, tB

        # ---- main loop (first chunks small to cut pipeline-fill latency)
        CHUNKS = [32, 96, 128, 128, 128]
        assert sum(CHUNKS) == TH
        m0 = 0
        for blk, CNT in enumerate(CHUNKS):
            xts = []
            for d, srcd in enumerate((xpf_d, xpb_d)):
                xb = xt_p.tile([128, 2, NT, NE], bf16, tag=f"xb{d}")
                for cc in range(2):
                    nc.sync.dma_start(
                        xb[:, cc, 0:CNT, :], srcd[cc][:, m0:m0 + CNT, :]
                    )
                xts.append(xb)
            # scr is [v, m, s]-ordered: writes carry the transpose cost
            # (overlapped with the matmuls, in row-groups of 8), the
            # inner-loop-critical plc read is 64 contiguous runs.
            scrb = scr_d[blk % 2].rearrange("v (m s) -> m v s", s=SW)
            for half in range(2):
                evb = evb_p.tile([NT, NE * SW], bf16, tag="evb")
                for vq in range(NE):
                    v = half * NE + vq
                    xb = xts[half]
                    ps = ps_p.tile([NT, SW], f32, tag="gps")
                    nc.tensor.matmul(
                        ps[0:CNT, :], xb[:, 0, 0:CNT, vq], sel_t[:, 0, v, :],
                        start=True, stop=False,
                    )
                    nc.tensor.matmul(
                        ps[0:CNT, :], xb[:, 1, 0:CNT, vq], sel_t[:, 1, v, :],
                        start=False, stop=True,
                    )
                    dst = evb[0:CNT, vq * SW:(vq + 1) * SW]
                    if blk == 0 and vq % 2 == 1:
                        # pre-loop: V is idle — split the PSUM copies so
                        # ScalarE isn't the chunk-0 serializer (GpSimd
                        # cannot access PSUM)
                        V.tensor_copy(dst, ps[0:CNT, :])
                    else:
                        S.copy(dst, ps[0:CNT, :])
                    if vq % 8 == 7:
                        v0 = half * NE + vq - 7
                        nc.sync.dma_start(
                            scrb[0:CNT, v0:v0 + 8, :],
                            evb[0:CNT, (vq - 7) * SW:(vq + 1) * SW],
                        )
            plc = chunk_p.tile([VR, NT * SW], bf16, tag="plc")
            nc.sync.dma_start(
                plc[:, 0:CNT * SW], scr_d[blk % 2][:, 0:CNT * SW]
            )
            for ml in range(CNT):
                m = m0 + ml
                pl = plc[:, ml * SW: ml * SW + 128]
                g0, D0 = cur_g, cur_D
                hN, kN, tN = new_h, new_k, new_t
                g1, D1 = new_g, new_D

                # V: 3 bf16 TT (4x) + 1 const-scalar STT; G: the pure D-add.
                # kh directly after h: runs before GpSimd's D1 saturates the
                # shared SBUF ports, and the D0-sem wait hides under it.
                V.tensor_tensor(hN[:, 2:130], g0[:, :], pl, Alu.mult)
                V.tensor_tensor(kN[:, 3:131], hN[:, 2:130], ksm_t[:, :], Alu.mult)
                V.scalar_tensor_tensor(
                    tN[:, :], D0[:, 0:128], float(np.exp(-CEQ)), hN[:, 2:130],
                    Alu.mult, Alu.add,
                )
                V.tensor_tensor(g1[:, :], tN[:, :], kN[:, 2:130], Alu.add)

                if m == TH - 1:
                    S.copy(utT[:, :], D0[:, :])   # u_T(bar) = D_(T-1)(bar)

                G.tensor_tensor(D1[:, :], D0[:, :], hN[:, 1:130], Alu.add)

                if (m + 1) % RB == 0:
                    r = (m + 1) // RB - 1
                    V.tensor_reduce(
                        rmax[:, r:r + 1], D1[:, :],
                        axis=mybir.AxisListType.X, op=Alu.max,
                    )
                    V.tensor_single_scalar(
                        rmax[:, r:r + 1], rmax[:, r:r + 1], 1e-30, Alu.max
                    )
                    V.reciprocal(rinv[:, :], rmax[:, r:r + 1])
                    V.tensor_single_scalar(g1[:, :], g1[:, :], rinv[:, 0:1], Alu.mult)
                    V.tensor_single_scalar(D1[:, :], D1[:, :], rinv[:, 0:1], Alu.mult)
                    if m == TH - 1:
                        V.tensor_single_scalar(
                            hN[:, 2:130], hN[:, 2:130], rinv[:, 0:1], Alu.mult
                        )
                        V.tensor_single_scalar(
                            utT[:, :], utT[:, :], rinv[:, 0:1], Alu.mult
                        )

                cur_g, new_g = new_g, cur_g
                cur_D, new_D = new_D, cur_D
                cur_h, new_h = new_h, cur_h
                cur_k, new_k = new_k, cur_k
                cur_t, new_t = new_t, cur_t
            m0 += CNT

        # ---- epilogue: log-space combine
        # fwd rows (0:NE): aL = h_final, aB = ut_final
        # bwd rows (NE:VR): bL = g_final, bB = D_final
        hF = cur_h    # last written h
        gF, DF = cur_g, cur_D

        hf32 = ep_p.tile([VR, L], f32, tag="hf32")
        uf32 = ep_p.tile([VR, 129], f32, tag="uf32")
        gf32 = ep_p.tile([VR, L], f32, tag="gf32")
        Df32 = ep_p.tile([VR, 129], f32, tag="Df32")
        S.copy(hf32[:, :], hF[:, 2:130])
        S.copy(uf32[:, :], utT[:, :])
        S.copy(gf32[:, :], gF[:, :])
        S.copy(Df32[:, :], DF[:, :])

        _sln_n = [0]

        def safe_ln(dst_ap, src_ap, n, rows=slice(0, VR)):
            """dst = ln(src) via exponent extraction; HW Ln table is only
            accurate on ~[1e-10, 2^64]; mantissa lives in [1, 2).
            Fresh scratch tiles per call so independent calls overlap."""
            _sln_n[0] += 1
            u = _sln_n[0]
            ii_t = ep_p.tile([VR, n], i32, tag=f"sln_i{u}")
            mm_t = ep_p.tile([VR, n], i32, tag=f"sln_m{u}")
            ee_t = ep_p.tile([VR, n], f32, tag=f"sln_e{u}")
            ll_t = ep_p.tile([VR, n], f32, tag=f"sln_l{u}")
            ii, mm, ee, ll = ii_t[rows, :], mm_t[rows, :], ee_t[rows, :], ll_t[rows, :]
            V.tensor_single_scalar(ii, src_ap.bitcast(i32), 23, Alu.arith_shift_right)
            V.tensor_single_scalar(ii, ii, 127, Alu.subtract)
            V.tensor_single_scalar(mm, src_ap.bitcast(i32), 0x007FFFFF, Alu.bitwise_and)
            V.tensor_single_scalar(mm, mm, 0x3F800000, Alu.bitwise_or)
            S.activation(ll, mm.bitcast(f32), Act.Ln)
            V.tensor_copy(ee, ii)
            V.scalar_tensor_tensor(
                dst_ap, ee, 0.6931471805599453, ll, Alu.mult, Alu.add
            )

        # lZ = sum of ln(rmax) over the 16 rescales
        lnr = ep_p.tile([VR, NR], f32, tag="lnr")
        lZ = ep_p.tile([VR, 1], f32, tag="lZ")
        safe_ln(lnr[:, :], rmax[:, :], NR)
        V.tensor_reduce(lZ[:, :], lnr[:, :], axis=mybir.AxisListType.X, op=Alu.add)
        V.tensor_tensor(lZ[:, :], lZ[:, :], lnpb_sum[:, :], Alu.add)

        LL = ep_p.tile([VR, L], f32, tag="LL")
        LB = ep_p.tile([VR, 129], f32, tag="LB")
        cl1 = ep_p.tile([VR, L], f32, tag="cl1")
        eq1 = ep_p.tile([VR, L], f32, tag="eq1")
        ln1 = ep_p.tile([VR, L], f32, tag="ln1")
        cl2 = ep_p.tile([VR, 129], f32, tag="cl2")
        eq2 = ep_p.tile([VR, 129], f32, tag="eq2")
        ln2 = ep_p.tile([VR, 129], f32, tag="ln2")

        # bwd rows first so the reversal DMAs start while fwd rows compute
        V.tensor_single_scalar(cl1[NE:VR, :], gf32[NE:VR, :], TINY, Alu.max)
        V.tensor_single_scalar(eq1[NE:VR, :], gf32[NE:VR, :], 0.0, Alu.is_equal)
        V.tensor_single_scalar(cl2[NE:VR, :], Df32[NE:VR, :], TINY, Alu.max)
        V.tensor_single_scalar(eq2[NE:VR, :], Df32[NE:VR, :], 0.0, Alu.is_equal)
        safe_ln(ln1[NE:VR, :], cl1[NE:VR, :], L, rows=slice(NE, VR))
        safe_ln(ln2[NE:VR, :], cl2[NE:VR, :], 129, rows=slice(NE, VR))
        V.scalar_tensor_tensor(
            LL[NE:VR, :], eq1[NE:VR, :], NEG, ln1[NE:VR, :], Alu.mult, Alu.add
        )
        V.scalar_tensor_tensor(
            LB[NE:VR, :], eq2[NE:VR, :], NEG, ln2[NE:VR, :], Alu.mult, Alu.add
        )
        V.tensor_single_scalar(LL[NE:VR, :], LL[NE:VR, :], lZ[NE:VR, 0:1], Alu.add)
        V.tensor_single_scalar(LB[NE:VR, :], LB[NE:VR, :], lZ[NE:VR, 0:1], Alu.add)

        LLb = ep_p.tile([NE, L], f32, tag="LLb")
        LBb = ep_p.tile([NE, 129], f32, tag="LBb")
        nc.sync.dma_start(LLb[:, :], LL[NE:VR, ::-1])
        nc.sync.dma_start(LBb[:, :], LB[NE:VR, ::-1])

        V.tensor_single_scalar(cl1[0:NE, :], hf32[0:NE, :], TINY, Alu.max)
        V.tensor_single_scalar(eq1[0:NE, :], hf32[0:NE, :], 0.0, Alu.is_equal)
        V.tensor_single_scalar(cl2[0:NE, :], uf32[0:NE, :], TINY, Alu.max)
        V.tensor_single_scalar(eq2[0:NE, :], uf32[0:NE, :], 0.0, Alu.is_equal)
        safe_ln(ln1[0:NE, :], cl1[0:NE, :], L, rows=slice(0, NE))
        safe_ln(ln2[0:NE, :], cl2[0:NE, :], 129, rows=slice(0, NE))
        V.scalar_tensor_tensor(
            LL[0:NE, :], eq1[0:NE, :], NEG, ln1[0:NE, :], Alu.mult, Alu.add
        )
        V.scalar_tensor_tensor(
            LB[0:NE, :], eq2[0:NE, :], NEG, ln2[0:NE, :], Alu.mult, Alu.add
        )
        V.tensor_single_scalar(LL[0:NE, :], LL[0:NE, :], lZ[0:NE, 0:1], Alu.add)
        V.tensor_single_scalar(LB[0:NE, :], LB[0:NE, :], lZ[0:NE, 0:1], Alu.add)

        lPL = ep_p.tile([NE, L], f32, tag="lPL")
        lPB = ep_p.tile([NE, 129], f32, tag="lPB")
        V.tensor_tensor(lPL[:, :], LL[0:NE, :], LLb[:, :], Alu.add)
        V.tensor_tensor(lPB[:, :], LB[0:NE, :], LBb[:, :], Alu.add)
        # equilibration pairing constants
        V.tensor_single_scalar(lPL[:, :], lPL[:, :], 127.0 * CEQ, Alu.add)
        V.tensor_single_scalar(lPB[:, :], lPB[:, :], 126.0 * CEQ, Alu.add)

        m1 = ep_p.tile([NE, 1], f32, tag="m1")
        m2 = ep_p.tile([NE, 1], f32, tag="m2")
        V.tensor_reduce(m1[:, :], lPL[:, :], axis=mybir.AxisListType.X, op=Alu.max)
        V.tensor_reduce(m2[:, :], lPB[:, :], axis=mybir.AxisListType.X, op=Alu.max)
        V.tensor_tensor(m1[:, :], m1[:, :], m2[:, :], Alu.max)
        sm1 = ep_p.tile([NE, L], f32, tag="sm1")
        sm2 = ep_p.tile([NE, 129], f32, tag="sm2")
        V.tensor_single_scalar(sm1[:, :], lPL[:, :], m1[:, 0:1], Alu.subtract)
        V.tensor_single_scalar(sm2[:, :], lPB[:, :], m1[:, 0:1], Alu.subtract)
        e1 = ep_p.tile([NE, L], f32, tag="e1")
        e2 = ep_p.tile([NE, 129], f32, tag="e2")
        s1 = ep_p.tile([NE, 1], f32, tag="s1")
        s2 = ep_p.tile([NE, 1], f32, tag="s2")
        S.activation(e1[:, :], sm1[:, :], Act.Exp, accum_out=s1[:, :])
        S.activation(e2[:, :], sm2[:, :], Act.Exp, accum_out=s2[:, :])
        V.tensor_tensor(s1[:, :], s1[:, :], s2[:, :], Alu.add)
        lsum = ep_p.tile([NE, 1], f32, tag="lsum")
        S.activation(lsum[:, :], s1[:, :], Act.Ln)
        lossT = ep_p.tile([NE, 1], f32, tag="lossT")
        V.tensor_tensor(lossT[:, :], m1[:, :], lsum[:, :], Alu.add)
        V.tensor_single_scalar(lossT[:, :], lossT[:, :], -1.0, Alu.mult)
        nc.sync.dma_start(loss_d[:, :], lossT[:, :])

    nc.compile()
    return nc


# ------------------------------- host side ------------------------------------

def make_inputs(y_true, y_pred):
    """Build the 8 per-core input maps (all bf16).

    Probs are pre-divided by the per-frame blank prob so the on-device D
    recursion is a pure add; sum(ln pb) is accounted via the pbln input.
    """
    y_true = np.asarray(y_true)
    y_pred = np.asarray(y_pred, dtype=np.float32)
    p = y_pred + np.float32(EPS)                           # [B, T, C]
    pb = p[:, :, C - 1:C]                                  # [B, T, 1]
    xq = (p / pb).astype(BF16NP)                           # ratio probs
    pbq = np.ascontiguousarray(pb[:, :, 0]).astype(BF16NP) # [B, T]

    lab = y_true.astype(np.int64)                          # [B, L]
    k = np.ones((B, L), np.float32)
    k[:, 1:] = (lab[:, 1:] != lab[:, :-1]).astype(np.float32)
    ks = np.ones((B, L), np.float32)
    ks[:, :-1] = k[:, 1:]
    e_c = np.float32(np.exp(-CEQ))

    cgrid = np.arange(128, dtype=np.int64)

    in_maps = []
    for q in range(NCORES):
        sl = slice(q * NE, (q + 1) * NE)
        pq = xq[sl]                                         # [NE, T, C]
        # layout [cc, c', t, e]: chunk DMAs become contiguous 8KB runs
        xpf = np.ascontiguousarray(
            pq[:, :TH, :].transpose(2, 1, 0).reshape(2, 128, TH, NE)
        )
        xpb = np.ascontiguousarray(
            pq[:, TH:, :][:, ::-1, :].transpose(2, 1, 0).reshape(2, 128, TH, NE)
        )

        pbln = np.empty((VR, TH), np.float32)
        pbln[:NE] = pbq[sl][:, :TH]
        pbln[NE:] = pbq[sl][:, TH:][:, ::-1]
        pbln = pbln.astype(BF16NP)

        labext = np.empty((VR, SW), np.int64)
        labext[:NE, :L] = lab[sl]
        labext[NE:, :L] = lab[sl][:, ::-1]
        # sel[c', cc, v, s] = (labext[v,s] == c' + 128*cc)
        sel = (
            labext[None, None, :, :] == (cgrid[:, None, None, None] + 128 * np.arange(2)[None, :, None, None])
        ).astype(np.float32)
        sel = np.ascontiguousarray(sel.reshape(128, 2 * VR * SW).astype(BF16NP))

        ksm = np.empty((VR, L), np.float32)
        ksm[:NE] = ks[sl]
        ksm[NE:] = k[sl][:, ::-1]
        ksm = (ksm * e_c).astype(BF16NP)

        in_maps.append({
            "xpf": xpf,
            "xpb": xpb,
            "sel": sel,
            "ksm": ksm,
            "pbln": pbln,
        })
    return in_maps


_NC_CACHE = {}


def _get_nc():
    if "nc" not in _NC_CACHE:
        _NC_CACHE["nc"] = build_nc()
    return _NC_CACHE["nc"]


def kernel(y_true, y_pred):
    _ensure_path()
    from concourse.bass_utils import run_bass_kernel_spmd

    nc = _get_nc()
    in_maps = make_inputs(y_true, y_pred)
    res = run_bass_kernel_spmd(nc, in_maps, core_ids=list(range(NCORES)))
    loss = np.concatenate([r["loss"] for r in res.results], axis=0)
    return loss.astype(np.float32)


if __name__ == "__main__":
    nc = build_nc()
    print("built + compiled OK")


# revision 25
# speedup vs baseline: 1.0104x; 1.0066x over previous
"""CTC loss (Keras ctc_batch_cost semantics) on 8 Trainium2 NeuronCores.

Strategy
--------
Data-parallel over batch: core q handles examples [32q, 32q+32).

The CTC alpha recursion runs in *linear probability space*, made feasible
for bf16/fp32 exponent range by three transforms:

1. Static exponential position equilibration: state stored as
   G[j] = g[j]*e^{-c*j}, D[i] = d[i]*e^{-c*(i-1)} with c = 3.5 (the
   measured lattice tilt in nats/position).  Folds entirely into the ksm
   mask, one scalar constant in the step, the D init value, and two
   scalar epilogue constants.
2. Blank-prob folding: the host pre-divides all probs by the per-frame
   blank prob, which turns the blank-chain update into a PURE ADD
   (D1 = D0 + shift(h)); sum(ln p_blank) is restored via one
   Ln-with-accumulate instruction over a [64, 512] input.
3. A single per-row floating rescale (by the D row max) every 64 steps.

Time is split forward/backward (meet in the middle): rows 0-31 run alpha
forward over t = 0..511, rows 32-63 run the suffix recursion backward
over t = 1023..512 in reversed label coordinates.

Per time step (bf16 state, 64 rows on partitions, ~129 free), emitted
in the order h, kh, t1, g1 so kh (a 4x op) runs before GpSimd's D1
saturates the SBUF ports DVE shares with it, and the D0 semaphore wait
hides under kh:
    h  = g0 * pl                   TT   DVE (4x mode)
    kh = h * ksm~                  TT   DVE (4x)
    t1 = (D0 * e^-c) + h           STT  DVE (const scalar, 1x)
    g1 = t1 + shift1(kh)           TT   DVE (4x)
    D1 = D0 + shift1(h)            TT   GpSimd (runs in parallel)
Shifts are contiguous APs via zero guard columns; guard offsets are
chosen so every DVE *read* starts at an even bf16 element (alignment
keeps the DVE fast mode).  Step period ~1.05us traced; the whole
recursion is the critical path, everything else overlaps it.

Per-step symbol probabilities are gathered on TensorE as one-hot matmuls
in bf16 (Sel matrices prebuilt on host, 128 label columns only), bounced
through DRAM to transpose [time-major] -> [row-major], all in bf16.
ScalarE does the PSUM->SBUF copies.  First chunks are small (32/96) to
cut pipeline-fill latency before the serial chain can start.
"""
import sys
from contextlib import ExitStack

import numpy as np
import ml_dtypes

# ---------------- problem constants (hardcoded per the task spec) -------------
B, T, C, L = 256, 1024, 256, 128
NCORES = 8
NE = B // NCORES          # examples per core (32)
VR = 2 * NE               # virtual rows: fwd + bwd (64)
TH = T // 2               # sequential steps per half (512)
NT = 128                  # time-block (chunk) size
NCHUNK = TH // NT         # 4
SW = 128                  # gathered columns: labels only (probs pre-divided by blank)
RB = 64                   # rescale interval
NR = TH // RB             # number of rescales (16)
EPS = 1e-7
CEQ = 3.5                 # equilibration nats/position
TINY = 1e-37
NEG = -1e18

_TRN_REPO = "/opt/trn_rl_repo"
BF16NP = ml_dtypes.bfloat16


def _ensure_path():
    if _TRN_REPO not in sys.path:
        sys.path.insert(0, _TRN_REPO)


# ----------------------------- device kernel ---------------------------------

def build_nc():
    """Build and compile the Bass module (same NEFF for all 8 cores)."""
    _ensure_path()
    import concourse.bass as bass
    import concourse.mybir as mybir
    import concourse.tile as tile
    from concourse import bacc

    f32 = mybir.dt.float32
    bf16 = mybir.dt.bfloat16
    i32 = mybir.dt.int32
    Alu = mybir.AluOpType
    Act = mybir.ActivationFunctionType

    nc = bacc.Bacc(
        "TRN2", target_bir_lowering=False, debug=False, num_devices=NCORES
    )

    xpf_d = nc.dram_tensor("xpf", [2, 128, TH, NE], bf16, kind="ExternalInput").ap()
    xpb_d = nc.dram_tensor("xpb", [2, 128, TH, NE], bf16, kind="ExternalInput").ap()
    sel_d = nc.dram_tensor("sel", [128, 2 * VR * SW], bf16, kind="ExternalInput").ap()
    ksm_d = nc.dram_tensor("ksm", [VR, L], bf16, kind="ExternalInput").ap()
    pbln_d = nc.dram_tensor("pbln", [VR, TH], bf16, kind="ExternalInput").ap()
    scr_d = nc.dram_tensor("scr", [2, VR, NT * SW], bf16).ap()
    loss_d = nc.dram_tensor("loss", [NE, 1], f32, kind="ExternalOutput").ap()

    with tile.TileContext(nc) as tc, ExitStack() as ctx:
        const_p = ctx.enter_context(tc.tile_pool(name="const", bufs=1))
        state_p = ctx.enter_context(tc.tile_pool(name="state", bufs=1))
        chunk_p = ctx.enter_context(tc.tile_pool(name="chunk", bufs=2))
        xt_p = ctx.enter_context(tc.tile_pool(name="xt", bufs=2))
        evb_p = ctx.enter_context(tc.tile_pool(name="evb", bufs=2))
        ps_p = ctx.enter_context(
            tc.tile_pool(name="ps", bufs=8, space=bass.MemorySpace.PSUM)
        )
        ep_p = ctx.enter_context(tc.tile_pool(name="ep", bufs=1))

        V, S, G = nc.vector, nc.scalar, nc.gpsimd

        # ---- constants (on the GpSimd-dispatched DMA queue so the sync
        # queue starts chunk-0's x loads immediately — one queue serializes)
        ksm_t = const_p.tile([VR, L], bf16, tag="ksm")
        G.dma_start(ksm_t[:, :], ksm_d[:, :])
        pbln_t = const_p.tile([VR, TH], bf16, tag="pbln")
        G.dma_start(pbln_t[:, :], pbln_d[:, :])
        lnpb = const_p.tile([VR, TH], f32, tag="lnpb")
        lnpb_sum = const_p.tile([VR, 1], f32, tag="lnpb_sum")
        S.activation(lnpb[:, :], pbln_t[:, :], Act.Ln, accum_out=lnpb_sum[:, :])
        sel_t = const_p.tile([128, 2, VR, SW], bf16, tag="sel")
        sel_src = sel_d.rearrange("c (x v s) -> c x v s", x=2, s=SW)
        # split the load so the first matmuls start after ~1/8 of the DMA
        for v0 in range(0, VR, 8):
            G.dma_start(
                sel_t[:, :, v0:v0 + 8, :], sel_src[:, :, v0:v0 + 8, :]
            )

        # ---- state tiles (bf16), ping-pong pairs
        def zt(tag, w):
            t = state_p.tile([VR, w], bf16, tag=tag)
            V.memset(t[:, :], 0.0)
            return t

        gA, gB = zt("gA", L), zt("gB", L)
        DA, DB = zt("DA", 129), zt("DB", 129)
        # h payload lives at cols 2-129 (even start), cols 0-1 = 0 guard;
        # kh payload at cols 3-130 (written shifted), cols 0-2 = 0 guard —
        # all V-engine READS start at even bf16 offsets (DVE fast mode).
        hA, hB = zt("hA", 130), zt("hB", 130)
        kA, kB = zt("kA", 131), zt("kB", 131)
        tA, tB = zt("tA", L), zt("tB", L)
        utT = state_p.tile([VR, 129], bf16, tag="utT")
        V.memset(gA[:, 0:1], 1.0)
        V.memset(DA[:, 0:1], float(np.exp(CEQ)))

        rmax = state_p.tile([VR, NR], f32, tag="rmax")
        rinv = state_p.tile([VR, 1], f32, tag="rinv")

        cur_g, new_g = gA, gB
        cur_D, new_D = DA, DB
        cur_h, new_h = hA, hB
        cur_k, new_k = kA, kB
        cur_t, new_t = tA, tB

        # ---- main loop (first chunks small to cut pipeline-fill latency)
        CHUNKS = [32, 96, 128, 128, 128]
        assert sum(CHUNKS) == TH
        m0 = 0
        for blk, CNT in enumerate(CHUNKS):
            xts = []
            for d, srcd in enumerate((xpf_d, xpb_d)):
                xb = xt_p.tile([128, 2, NT, NE], bf16, tag=f"xb{d}")
                for cc in range(2):
                    nc.sync.dma_start(
                        xb[:, cc, 0:CNT, :], srcd[cc][:, m0:m0 + CNT, :]
                    )
                xts.append(xb)
            # scr is [v, m, s]-ordered: writes carry the transpose cost
            # (overlapped with the matmuls, in row-groups of 8), the
            # inner-loop-critical plc read is 64 contiguous runs.
            scrb = scr_d[blk % 2].rearrange("v (m s) -> m v s", s=SW)
            for half in range(2):
                evb = evb_p.tile([NT, NE * SW], bf16, tag="evb")
                for vq in range(NE):
                    v = half * NE + vq
                    xb = xts[half]
                    ps = ps_p.tile([NT, SW], f32, tag="gps")
                    nc.tensor.matmul(
                        ps[0:CNT, :], xb[:, 0, 0:CNT, vq], sel_t[:, 0, v, :],
                        start=True, stop=False,
                    )
                    nc.tensor.matmul(
                        ps[0:CNT, :], xb[:, 1, 0:CNT, vq], sel_t[:, 1, v, :],
                        start=False, stop=True,
                    )
                    dst = evb[0:CNT, vq * SW:(vq + 1) * SW]
                    if blk == 0 and vq % 2 == 1:
                        # pre-loop: V is idle — split the PSUM copies so
                        # ScalarE isn't the chunk-0 serializer (GpSimd
                        # cannot access PSUM)
                        V.tensor_copy(dst, ps[0:CNT, :])
                    else:
                        S.copy(dst, ps[0:CNT, :])
                    if vq % 8 == 7:
                        v0 = half * NE + vq - 7
                        nc.sync.dma_start(
                            scrb[0:CNT, v0:v0 + 8, :],
                            evb[0:CNT, (vq - 7) * SW:(vq + 1) * SW],
                        )
            plc = chunk_p.tile([VR, NT * SW], bf16, tag="plc")
            nc.sync.dma_start(
                plc[:, 0:CNT * SW], scr_d[blk % 2][:, 0:CNT * SW]
            )
            for ml in range(CNT):
                m = m0 + ml
                pl = plc[:, ml * SW: ml * SW + 128]
                g0, D0 = cur_g, cur_D
                hN, kN, tN = new_h, new_k, new_t
                g1, D1 = new_g, new_D

                # All 5 ops on DVE: independent consecutive ops dual-issue
                # (~80ns overlap), and dropping GpSimd removes the shared
                # SBUF-port contention entirely.  D1 sits between t1 and g1
                # (independent of both) so it pipelines with the pair.
                V.tensor_tensor(hN[:, 2:130], g0[:, :], pl, Alu.mult)
                V.tensor_tensor(kN[:, 3:131], hN[:, 2:130], ksm_t[:, :], Alu.mult)
                V.scalar_tensor_tensor(
                    tN[:, :], D0[:, 0:128], float(np.exp(-CEQ)), hN[:, 2:130],
                    Alu.mult, Alu.add,
                )
                V.scalar_tensor_tensor(
                    D1[:, :], D0[:, :], 1.0, hN[:, 1:130], Alu.mult, Alu.add
                )
                V.tensor_tensor(g1[:, :], tN[:, :], kN[:, 2:130], Alu.add)

                if m == TH - 1:
                    S.copy(utT[:, :], D0[:, :])   # u_T(bar) = D_(T-1)(bar)

                if (m + 1) % RB == 0:
                    r = (m + 1) // RB - 1
                    V.tensor_reduce(
                        rmax[:, r:r + 1], D1[:, :],
                        axis=mybir.AxisListType.X, op=Alu.max,
                    )
                    V.tensor_single_scalar(
                        rmax[:, r:r + 1], rmax[:, r:r + 1], 1e-30, Alu.max
                    )
                    V.reciprocal(rinv[:, :], rmax[:, r:r + 1])
                    V.tensor_single_scalar(g1[:, :], g1[:, :], rinv[:, 0:1], Alu.mult)
                    V.tensor_single_scalar(D1[:, :], D1[:, :], rinv[:, 0:1], Alu.mult)
                    if m == TH - 1:
                        V.tensor_single_scalar(
                            hN[:, 2:130], hN[:, 2:130], rinv[:, 0:1], Alu.mult
                        )
                        V.tensor_single_scalar(
                            utT[:, :], utT[:, :], rinv[:, 0:1], Alu.mult
                        )

                cur_g, new_g = new_g, cur_g
                cur_D, new_D = new_D, cur_D
                cur_h, new_h = new_h, cur_h
                cur_k, new_k = new_k, cur_k
                cur_t, new_t = new_t, cur_t
            m0 += CNT

        # ---- epilogue: log-space combine
        # fwd rows (0:NE): aL = h_final, aB = ut_final
        # bwd rows (NE:VR): bL = g_final, bB = D_final
        hF = cur_h    # last written h
        gF, DF = cur_g, cur_D

        hf32 = ep_p.tile([VR, L], f32, tag="hf32")
        uf32 = ep_p.tile([VR, 129], f32, tag="uf32")
        gf32 = ep_p.tile([VR, L], f32, tag="gf32")
        Df32 = ep_p.tile([VR, 129], f32, tag="Df32")
        S.copy(hf32[:, :], hF[:, 2:130])
        S.copy(uf32[:, :], utT[:, :])
        S.copy(gf32[:, :], gF[:, :])
        S.copy(Df32[:, :], DF[:, :])

        _sln_n = [0]

        def safe_ln(dst_ap, src_ap, n, rows=slice(0, VR)):
            """dst = ln(src) via exponent extraction; HW Ln table is only
            accurate on ~[1e-10, 2^64]; mantissa lives in [1, 2).
            Fresh scratch tiles per call so independent calls overlap."""
            _sln_n[0] += 1
            u = _sln_n[0]
            ii_t = ep_p.tile([VR, n], i32, tag=f"sln_i{u}")
            mm_t = ep_p.tile([VR, n], i32, tag=f"sln_m{u}")
            ee_t = ep_p.tile([VR, n], f32, tag=f"sln_e{u}")
            ll_t = ep_p.tile([VR, n], f32, tag=f"sln_l{u}")
            ii, mm, ee, ll = ii_t[rows, :], mm_t[rows, :], ee_t[rows, :], ll_t[rows, :]
            V.tensor_single_scalar(ii, src_ap.bitcast(i32), 23, Alu.arith_shift_right)
            V.tensor_single_scalar(ii, ii, 127, Alu.subtract)
            V.tensor_single_scalar(mm, src_ap.bitcast(i32), 0x007FFFFF, Alu.bitwise_and)
            V.tensor_single_scalar(mm, mm, 0x3F800000, Alu.bitwise_or)
            S.activation(ll, mm.bitcast(f32), Act.Ln)
            V.tensor_copy(ee, ii)
            V.scalar_tensor_tensor(
                dst_ap, ee, 0.6931471805599453, ll, Alu.mult, Alu.add
            )

        # lZ = sum of ln(rmax) over the 16 rescales
        lnr = ep_p.tile([VR, NR], f32, tag="lnr")
        lZ = ep_p.tile([VR, 1], f32, tag="lZ")
        safe_ln(lnr[:, :], rmax[:, :], NR)
        V.tensor_reduce(lZ[:, :], lnr[:, :], axis=mybir.AxisListType.X, op=Alu.add)
        V.tensor_tensor(lZ[:, :], lZ[:, :], lnpb_sum[:, :], Alu.add)

        LL = ep_p.tile([VR, L], f32, tag="LL")
        LB = ep_p.tile([VR, 129], f32, tag="LB")
        cl1 = ep_p.tile([VR, L], f32, tag="cl1")
        eq1 = ep_p.tile([VR, L], f32, tag="eq1")
        ln1 = ep_p.tile([VR, L], f32, tag="ln1")
        cl2 = ep_p.tile([VR, 129], f32, tag="cl2")
        eq2 = ep_p.tile([VR, 129], f32, tag="eq2")
        ln2 = ep_p.tile([VR, 129], f32, tag="ln2")

        # bwd rows first so the reversal DMAs start while fwd rows compute
        V.tensor_single_scalar(cl1[NE:VR, :], gf32[NE:VR, :], TINY, Alu.max)
        V.tensor_single_scalar(eq1[NE:VR, :], gf32[NE:VR, :], 0.0, Alu.is_equal)
        V.tensor_single_scalar(cl2[NE:VR, :], Df32[NE:VR, :], TINY, Alu.max)
        V.tensor_single_scalar(eq2[NE:VR, :], Df32[NE:VR, :], 0.0, Alu.is_equal)
        safe_ln(ln1[NE:VR, :], cl1[NE:VR, :], L, rows=slice(NE, VR))
        safe_ln(ln2[NE:VR, :], cl2[NE:VR, :], 129, rows=slice(NE, VR))
        V.scalar_tensor_tensor(
            LL[NE:VR, :], eq1[NE:VR, :], NEG, ln1[NE:VR, :], Alu.mult, Alu.add
        )
        V.scalar_tensor_tensor(
            LB[NE:VR, :], eq2[NE:VR, :], NEG, ln2[NE:VR, :], Alu.mult, Alu.add
        )
        V.tensor_single_scalar(LL[NE:VR, :], LL[NE:VR, :], lZ[NE:VR, 0:1], Alu.add)
        V.tensor_single_scalar(LB[NE:VR, :], LB[NE:VR, :], lZ[NE:VR, 0:1], Alu.add)

        LLb = ep_p.tile([NE, L], f32, tag="LLb")
        LBb = ep_p.tile([NE, 129], f32, tag="LBb")
        nc.sync.dma_start(LLb[:, :], LL[NE:VR, ::-1])
        nc.sync.dma_start(LBb[:, :], LB[NE:VR, ::-1])

        V.tensor_single_scalar(cl1[0:NE, :], hf32[0:NE, :], TINY, Alu.max)
        V.tensor_single_scalar(eq1[0:NE, :], hf32[0:NE, :], 0.0, Alu.is_equal)
        V.tensor_single_scalar(cl2[0:NE, :], uf32[0:NE, :], TINY, Alu.max)
        V.tensor_single_scalar(eq2[0:NE, :], uf32[0:NE, :], 0.0, Alu.is_equal)
        safe_ln(ln1[0:NE, :], cl1[0:NE, :], L, rows=slice(0, NE))
        safe_ln(ln2[0:NE, :], cl2[0:NE, :], 129, rows=slice(0, NE))
        V.scalar_tensor_tensor(
            LL[0:NE, :], eq1[0:NE, :], NEG, ln1[0:NE, :], Alu.mult, Alu.add
        )
        V.scalar_tensor_tensor(
            LB[0:NE, :], eq2[0:NE, :], NEG, ln2[0:NE, :], Alu.mult, Alu.add
        )
        V.tensor_single_scalar(LL[0:NE, :], LL[0:NE, :], lZ[0:NE, 0:1], Alu.add)
        V.tensor_single_scalar(LB[0:NE, :], LB[0:NE, :], lZ[0:NE, 0:1], Alu.add)

        lPL = ep_p.tile([NE, L], f32, tag="lPL")
        lPB = ep_p.tile([NE, 129], f32, tag="lPB")
        V.tensor_tensor(lPL[:, :], LL[0:NE, :], LLb[:, :], Alu.add)
        V.tensor_tensor(lPB[:, :], LB[0:NE, :], LBb[:, :], Alu.add)
        # equilibration pairing constants
        V.tensor_single_scalar(lPL[:, :], lPL[:, :], 127.0 * CEQ, Alu.add)
        V.tensor_single_scalar(lPB[:, :], lPB[:, :], 126.0 * CEQ, Alu.add)

        m1 = ep_p.tile([NE, 1], f32, tag="m1")
        m2 = ep_p.tile([NE, 1], f32, tag="m2")
        V.tensor_reduce(m1[:, :], lPL[:, :], axis=mybir.AxisListType.X, op=Alu.max)
        V.tensor_reduce(m2[:, :], lPB[:, :], axis=mybir.AxisListType.X, op=Alu.max)
        V.tensor_tensor(m1[:, :], m1[:, :], m2[:, :], Alu.max)
        sm1 = ep_p.tile([NE, L], f32, tag="sm1")
        sm2 = ep_p.tile([NE, 129], f32, tag="sm2")
        V.tensor_single_scalar(sm1[:, :], lPL[:, :], m1[:, 0:1], Alu.subtract)
        V.tensor_single_scalar(sm2[:, :], lPB[:, :], m1[:, 0:1], Alu.subtract)
        e1 = ep_p.tile([NE, L], f32, tag="e1")
        e2 = ep_p.tile([NE, 129], f32, tag="e2")
        s1 = ep_p.tile([NE, 1], f32, tag="s1")
        s2 = ep_p.tile([NE, 1], f32, tag="s2")
        S.activation(e1[:, :], sm1[:, :], Act.Exp, accum_out=s1[:, :])
        S.activation(e2[:, :], sm2[:, :], Act.Exp, accum_out=s2[:, :])
        V.tensor_tensor(s1[:, :], s1[:, :], s2[:, :], Alu.add)
        lsum = ep_p.tile([NE, 1], f32, tag="lsum")
        S.activation(lsum[:, :], s1[:, :], Act.Ln)
        lossT = ep_p.tile([NE, 1], f32, tag="lossT")
        V.tensor_tensor(lossT[:, :], m1[:, :], lsum[:, :], Alu.add)
        V.tensor_single_scalar(lossT[:, :], lossT[:, :], -1.0, Alu.mult)
        nc.sync.dma_start(loss_d[:, :], lossT[:, :])

    nc.compile()
    return nc


# ------------------------------- host side ------------------------------------

def make_inputs(y_true, y_pred):
    """Build the 8 per-core input maps (all bf16).

    Probs are pre-divided by the per-frame blank prob so the on-device D
    recursion is a pure add; sum(ln pb) is accounted via the pbln input.
    """
    y_true = np.asarray(y_true)
    y_pred = np.asarray(y_pred, dtype=np.float32)
    p = y_pred + np.float32(EPS)                           # [B, T, C]
    pb = p[:, :, C - 1:C]                                  # [B, T, 1]
    xq = (p / pb).astype(BF16NP)                           # ratio probs
    pbq = np.ascontiguousarray(pb[:, :, 0]).astype(BF16NP) # [B, T]

    lab = y_true.astype(np.int64)                          # [B, L]
    k = np.ones((B, L), np.float32)
    k[:, 1:] = (lab[:, 1:] != lab[:, :-1]).astype(np.float32)
    ks = np.ones((B, L), np.float32)
    ks[:, :-1] = k[:, 1:]
    e_c = np.float32(np.exp(-CEQ))

    cgrid = np.arange(128, dtype=np.int64)

    in_maps = []
    for q in range(NCORES):
        sl = slice(q * NE, (q + 1) * NE)
        pq = xq[sl]                                         # [NE, T, C]
        # layout [cc, c', t, e]: chunk DMAs become contiguous 8KB runs
        xpf = np.ascontiguousarray(
            pq[:, :TH, :].transpose(2, 1, 0).reshape(2, 128, TH, NE)
        )
        xpb = np.ascontiguousarray(
            pq[:, TH:, :][:, ::-1, :].transpose(2, 1, 0).reshape(2, 128, TH, NE)
        )

        pbln = np.empty((VR, TH), np.float32)
        pbln[:NE] = pbq[sl][:, :TH]
        pbln[NE:] = pbq[sl][:, TH:][:, ::-1]
        pbln = pbln.astype(BF16NP)

        labext = np.empty((VR, SW), np.int64)
        labext[:NE, :L] = lab[sl]
        labext[NE:, :L] = lab[sl][:, ::-1]
        # sel[c', cc, v, s] = (labext[v,s] == c' + 128*cc)
        sel = (
            labext[None, None, :, :] == (cgrid[:, None, None, None] + 128 * np.arange(2)[None, :, None, None])
        ).astype(np.float32)
        sel = np.ascontiguousarray(sel.reshape(128, 2 * VR * SW).astype(BF16NP))

        ksm = np.empty((VR, L), np.float32)
        ksm[:NE] = ks[sl]
        ksm[NE:] = k[sl][:, ::-1]
        ksm = (ksm * e_c).astype(BF16NP)

        in_maps.append({
            "xpf": xpf,
            "xpb": xpb,
            "sel": sel,
            "ksm": ksm,
            "pbln": pbln,
        })
    return in_maps


_NC_CACHE = {}


def _get_nc():
    if "nc" not in _NC_CACHE:
        _NC_CACHE["nc"] = build_nc()
    return _NC_CACHE["nc"]


def kernel(y_true, y_pred):
    _ensure_path()
    from concourse.bass_utils import run_bass_kernel_spmd

    nc = _get_nc()
    in_maps = make_inputs(y_true, y_pred)
    res = run_bass_kernel_spmd(nc, in_maps, core_ids=list(range(NCORES)))
    loss = np.concatenate([r["loss"] for r in res.results], axis=0)
    return loss.astype(np.float32)


if __name__ == "__main__":
    nc = build_nc()
    print("built + compiled OK")
